# revision 16
# baseline (speedup 1.0000x reference)
"""Trainium2 Bass kernel for nn_AttachmentPredictor.

Computation (per batch row b):
  head = x[b, :-2, :] @ proj_head + bias_b,  bias_b = proj_prep.T @ x[b,-2]
                                           + proj_child.T @ x[b,-1]
  composed = tanh(head)                      # [T-2, P]
  composed = tanh(composed @ hidden_W[0])
  composed = tanh(composed @ hidden_W[1])
  scores = composed @ scorer                 # [T-2]
  out = where(mask, exp(scores), 0); out /= (sum(out) + 1e-7)

Sharding: pure data parallel, batch 64 -> 8 rows per core on 8 cores.

Masked-out tokens contribute exactly zero to the output, so the host gathers
each row's masked-in tokens into a compact layout (rows sorted by count so
each slot pads only to its own 16-multiple), the device runs the dense
pipeline on compacted tokens, and the host scatters results back.

Device layout: all activations transposed [P on partitions, tokens free].
The host pre-transposes x (and w1) into the on-chip layout so every
transfer is a plain DMACopy - the cost model's DGE pre-stages same-type
descriptors back-to-back, while copy<->xbar-transpose switches serialize
on full DMA completion (~2.2us each).  All GEMMs run in bf16.

Cost-model-driven schedule notes:
 - The DGE admits only ~2 descriptors in flight (a descriptor is staged
   ~900ns after the transfer two back completes), so small DMAs cost
   ~1.5-2.2us each regardless of size.  Everything small rides in ONE
   combined tensor (wpcx = wp | wc | prep/child columns | scorer column |
   mask-penalty columns); h0/h1 share one tensor; DMA sources keep >=512B
   contiguous runs (below that the cost model halves DMA bandwidth).
 - wpcx goes FIRST: the per-row bias vectors (computed on the PE in the
   prologue) gate every layer-1 tanh, so they must exist before the first
   row's tanh.  Row 0's x follows in chunk-sized pieces so layer 1 starts
   at ~11.5us; later rows take one whole-row DMA each, issued a row ahead.
 - The PE p-state ramp (2x slower for the first 3us of a continuous run)
   makes PE idle gaps doubly expensive; warmup dummy matmuls keep the PE
   continuously busy through the DMA-bound prologue.
 - A dummy Exp in the prologue pulls the 1283ns activation-table load
   (exp_and_others covers both Tanh and Exp) off the mid-kernel path.
 - Masked softmax via additive penalty: every score-PSUM region is opened
   by a start=True identity-matmul writing 0 (valid) / -40 (masked or
   padding) from the wpcx penalty columns; the scorer matmuls then
   accumulate onto it.  exp yields masked_exp directly and its accum_out
   port produces the per-partition row sum in the same instruction; one
   ones-matmul sums over partitions and broadcasts.  The tail has no PE
   transpose and almost no work; its pieces are deferred into the next
   row's instruction stream so the in-order queues never stall.
 - The last row ends with a small chunk (paired from the end) so the
   exposed end-of-kernel tanh->score->exp->normalize->DMA chain is short.
"""

import sys

import numpy as np

sys.path.insert(0, "/opt/trn_rl_repo")

B = 64
T = 2048
TH = 2046  # head tokens
D = 1024
P = 512
NCORES = 8
R = B // NCORES  # 8 batch rows per core
KD = D // 128  # 8 contraction chunks for layer 1
KP = P // 128  # 4 contraction chunks for layers 2/3/scorer
J16 = 16  # score blocks in the (zero-padded) tail
XCOL = 2 * P  # wpcx column offset of prep/child columns
SCOL = 2 * P + 2 * R  # wpcx column offset of the scorer column
PCOL = SCOL + 1  # wpcx column offset of the mask-penalty columns
WPCX_COLS = PCOL + J16
PEN = -40.0  # additive mask penalty: exp(-40) ~ 4e-18 ~ 0
WARM_A = 21  # prologue warmup matmuls before the bias matmuls
WARM_B = 7  # warmup matmuls between bias and row 0's layer 1

_CACHE = {}


def _chunks(PADT):
    """Token-chunk lengths covering PADT.  Chunks are at most 256 tokens so
    a fused [128, KP, L] mid-layer PSUM tile fits in 2 banks, and must
    start on a 128 boundary unless they fit inside one 128-token score
    block."""
    out = []
    while PADT > 0:
        c = min(256, PADT)
        out.append(c)
        PADT -= c
    return out


_chunks_first = _chunks


def _chunks_last(PADT):
    """The final row ends with a small chunk so the exposed end-of-kernel
    tanh->score->softmax chain is short.  The small chunk must start at a
    partition offset of 0/32/64 within its 128-token score block (matmul
    output base-partition constraint) and not straddle a block boundary."""
    if PADT < 640:
        return _chunks(PADT)
    for small in (48, 64, 80, 96):
        po = (PADT - small) % 128
        if po in (0, 32, 64) and po + small <= 128:
            return _chunks(PADT - small) + [small]
    return _chunks(PADT)


def _groups(chs, pair_from_end=False):
    n = len(chs)
    if not pair_from_end or n % 2 == 0:
        return [
            tuple(g for g in (2 * i, 2 * i + 1) if g < n)
            for i in range((n + 1) // 2)
        ]
    return [(0,)] + [(2 * i + 1, 2 * i + 2) for i in range((n - 1) // 2)]


def _build(padts):
    import concourse.bass as bass
    import concourse.mybir as mybir
    import concourse.tile as tile
    from concourse import bacc
    from concourse.masks import make_identity

    f32 = mybir.dt.float32
    bf16 = mybir.dt.bfloat16
    AF = mybir.ActivationFunctionType

    TCS = list(padts)
    CHSS = [_chunks_first(padts[0])]
    CHSS += [_chunks(p) for p in padts[1:-1]]
    CHSS.append(_chunks_last(padts[-1]))
    OFFS = [[sum(chs[:c]) for c in range(len(chs))] for chs in CHSS]
    GRPS = [
        _groups(chs, pair_from_end=(r == R - 1)) for r, chs in enumerate(CHSS)
    ]
    TCMAX = max(TCS)

    nc = bacc.Bacc(
        "TRN2", target_bir_lowering=False, debug=False, num_devices=NCORES
    )

    # x pre-transposed on host: xs[r, p, k*TCMAX + t] = x_compact[r, t, k*128+p]
    xs = nc.dram_tensor(
        "xs", [R, 128, KD * TCMAX], bf16, kind="ExternalInput"
    ).ap()
    # w1 pre-transposed m-major: w1m[p, m, k, q] = w1[k*128+p, m*128+q]
    w1m = nc.dram_tensor(
        "w1m", [128, KP, KD, 128], bf16, kind="ExternalInput"
    ).ap()
    wpcx = nc.dram_tensor(
        "wpcx", [D, WPCX_COLS], bf16, kind="ExternalInput"
    ).ap()
    hh = nc.dram_tensor("hh", [2 * P, P], bf16, kind="ExternalInput").ap()
    out = nc.dram_tensor("out", [R, 128, J16], f32, kind="ExternalOutput").ap()

    with tile.TileContext(nc) as tc:
        with (
            tc.tile_pool(name="l1p_pool", bufs=2, space="PSUM") as l1p_pool,
            tc.tile_pool(name="midp_pool", bufs=2, space="PSUM") as midp_pool,
            tc.tile_pool(name="scp_pool", bufs=2, space="PSUM") as scp_pool,
            tc.tile_pool(name="wpool", bufs=1) as wpool,
            tc.tile_pool(name="cpool", bufs=1) as cpool,
            tc.tile_pool(name="xt_pool", bufs=2) as xt_pool,
            tc.tile_pool(name="y_pool", bufs=2 * KP) as y_pool,
            tc.tile_pool(name="tail_pool", bufs=2) as tail_pool,
        ):
            # ---- transposed x tiles, one per row: xt[p, k, t]
            xts = {}

            def issue_xt_row(r):
                """Whole-row x DMA (rows >= 1), issued a row ahead."""
                xts[r] = xt_pool.tile(
                    [128, KD, TCS[r]], bf16, tag="xtr", name=f"xt{r}"
                )
                nc.sync.dma_start(
                    xts[r][:],
                    xs[r, :, :].rearrange("p (k l) -> p k l", k=KD)[
                        :, :, 0 : TCS[r]
                    ],
                )

            # ---- prologue DMAs: wpcx (bias inputs) first, then w1 +
            # row-0 x in chunk-sized pieces, then h0/h1, then row 1.
            w1t = wpool.tile([128, KP, KD, 128], bf16)
            wpcxt = wpool.tile([128, KD, WPCX_COLS], bf16)
            hht = wpool.tile([128, 2 * KP, P], bf16)
            xts[0] = xt_pool.tile(
                [128, KD, TCS[0]], bf16, tag="xtr", name="xt0"
            )

            def issue_x0_chunk(c):
                t0, L = OFFS[0][c], CHSS[0][c]
                nc.sync.dma_start(
                    xts[0][:, :, t0 : t0 + L],
                    xs[0, :, :].rearrange("p (k l) -> p k l", k=KD)[
                        :, :, t0 : t0 + L
                    ],
                )

            nc.sync.dma_start(
                wpcxt[:], wpcx.rearrange("(k p) q -> p k q", p=128)
            )
            nc.sync.dma_start(w1t[:, 0:1, :, :], w1m[:, 0:1, :, :])
            issue_x0_chunk(0)
            nc.sync.dma_start(w1t[:, 1:KP, :, :], w1m[:, 1:KP, :, :])
            issue_x0_chunk(1)
            nc.sync.dma_start(hht[:], hh.rearrange("(k p) q -> p k q", p=128))
            for c in range(2, len(CHSS[0])):
                issue_x0_chunk(c)

            wpt = wpcxt[:, :, 0:P]
            wct = wpcxt[:, :, P:XCOL]
            h0t = hht[:, 0:KP, :]
            h1t = hht[:, KP : 2 * KP, :]

            z0 = cpool.tile([128, 512], bf16)
            nc.gpsimd.memset(z0[:], 0.0)
            ones128 = cpool.tile([128, 128], f32)
            nc.vector.memset(ones128[:], 1.0)
            identb = cpool.tile([128, 128], bf16)
            make_identity(nc, identb[:])
            biasT = cpool.tile([128, KP, R], f32)

            # Dummy Exp: forces the single exp_and_others table load (covers
            # Tanh AND Exp) into the idle prologue Activation stream.
            e_warm = tail_pool.tile([128, J16], f32, tag="esb", name="e_warm")
            nc.scalar.activation(e_warm[:], z0[:, 0:J16], AF.Exp)

            # PE warm-up + ramp keeper: the tensor engine only reaches full
            # clock after ~3us of continuous execution; dummy matmuls keep it
            # busy (and the ramp hot) wherever the schedule would stall.
            warm_i = [0]

            def warm(n, length=256):
                for _ in range(n):
                    dmy = l1p_pool.tile(
                        [128, length], f32, tag="mm", name=f"wm{warm_i[0]}"
                    )
                    warm_i[0] += 1
                    nc.tensor.matmul(dmy[:], z0[:, 0:128], z0[:, 0:length])

            def emit_bias():
                """All R bias vectors: bias_ps[:, m, r] = wp.T @ prep_r +
                wc.T @ child_r, accumulated per PSUM column; one DVE copy
                moves everything to SBUF."""
                bias_ps = l1p_pool.tile(
                    [128, KP, R], f32, tag="mm", name="bps"
                )
                for r in range(R):
                    for m in range(KP):
                        mb = slice(m * 128, (m + 1) * 128)
                        for k in range(KD):
                            nc.tensor.matmul(
                                bias_ps[:, m, r : r + 1],
                                wpt[:, k, mb],
                                wpcxt[:, k, XCOL + 2 * r : XCOL + 2 * r + 1],
                                start=(k == 0),
                                stop=False,
                            )
                        for k in range(KD):
                            nc.tensor.matmul(
                                bias_ps[:, m, r : r + 1],
                                wct[:, k, mb],
                                wpcxt[
                                    :, k, XCOL + 2 * r + 1 : XCOL + 2 * r + 2
                                ],
                                start=False,
                                stop=(k == KD - 1),
                            )
                nc.vector.tensor_copy(biasT[:], bias_ps[:])

            # ---- helpers -------------------------------------------------
            def emit_l1(r, c, ys):
                t0, L = OFFS[r][c], CHSS[r][c]
                for m in range(KP):
                    ps = l1p_pool.tile([128, L], f32, tag="mm", name="l1ps")
                    for k in range(KD):
                        nc.tensor.matmul(
                            ps[:],
                            w1t[:, m, k, :],
                            xts[r][:, k, t0 : t0 + L],
                            start=(k == 0),
                            stop=(k == KD - 1),
                        )
                    y = y_pool.tile([128, L], bf16, tag="y1", name="y1")
                    nc.scalar.activation(
                        y[:], ps[:], AF.Tanh, bias=biasT[:, m, r : r + 1]
                    )
                    ys[(c, m)] = y

            def emit_mid(wt, yget, r, c, ys, ytag):
                # fused mid layer: all KP m-blocks accumulate into one
                # [128, KP, L] PSUM tile (2 banks) and get ONE tanh.
                L = CHSS[r][c]
                ps = midp_pool.tile([128, KP, L], f32, tag="mid", name="lps")
                for m in range(KP):
                    mb = slice(m * 128, (m + 1) * 128)
                    for k in range(KP):
                        nc.tensor.matmul(
                            ps[:, m, :],
                            wt[:, k, mb],
                            yget(c, k),
                            start=(k == 0),
                            stop=(k == KP - 1),
                        )
                y = y_pool.tile([128, KP, L], bf16, tag=ytag, name=ytag)
                nc.scalar.activation(y[:], ps[:], AF.Tanh)
                ys[c] = y

            def emit_score(sc_ps, y3s, r, c):
                t0, L = OFFS[r][c], CHSS[r][c]
                done = 0
                while done < L:
                    t = t0 + done
                    col = t // 128
                    po = t % 128
                    w = min(128 - po, L - done)
                    jb = slice(done, done + w)
                    # open the accumulation group with the mask penalty (a
                    # start=True matmul marks the whole PSUM bank
                    # pending-zero, so every region must begin with its own
                    # start=True write; the scorer then accumulates)
                    nc.tensor.matmul(
                        sc_ps[po : po + w, col : col + 1],
                        identb[:, po : po + w],
                        wpcxt[:, r, PCOL + col : PCOL + col + 1],
                        start=True,
                        stop=False,
                    )
                    for k in range(KP):
                        nc.tensor.matmul(
                            sc_ps[po : po + w, col : col + 1],
                            y3s[c][:, k, jb],
                            wpcxt[:, k, SCOL : SCOL + 1],
                            start=False,
                            stop=(k == KP - 1),
                        )
                    done += w

            # ---- per-row masked-softmax tail.  exp of (scores + penalty)
            # gives masked_exp directly; its accumulator port the
            # per-partition sum.  Pieces run in the NEXT row's instruction
            # stream so in-order queues never stall.
            tails = {}

            def init_scps(r):
                sc_ps = scp_pool.tile(
                    [128, J16], f32, tag="scps", name="sc_ps"
                )
                tails[r] = {"sc_ps": sc_ps}
                return sc_ps

            def emit_basecoat(r, sc_ps):
                # base-coat: every score column gets its penalty via a
                # start=True matmul, covering never-scored columns and the
                # unscored partitions of the last partial column.  Emitted
                # just before the row's first scores so the PE queue does not
                # wait on the previous row's exp (ring WAR).
                for j in range(J16):
                    nc.tensor.matmul(
                        sc_ps[:, j : j + 1],
                        identb[:],
                        wpcxt[:, r, PCOL + j : PCOL + j + 1],
                        start=True,
                        stop=True,
                    )

            def tail_exp(r):
                st = tails[r]
                e2 = tail_pool.tile([128, J16], f32, tag="esb", name="e2")
                rs = tail_pool.tile([128, 1], f32, tag="rs", name="rs")
                nc.scalar.activation(
                    e2[:], st["sc_ps"][:], AF.Exp, accum_out=rs[:]
                )
                st["e2"] = e2
                st["rs"] = rs

            def tail_sum(r):
                # ones.T @ rs: sums rs over all 128 partitions and broadcasts
                # the total back to 128 partitions, in one ap-1 matmul.
                st = tails[r]
                sb = scp_pool.tile([128, 1], f32, tag="scps", name="sb")
                nc.tensor.matmul(sb[:], ones128[:], st["rs"][:])
                st["sb"] = sb

            def tail_norm(r):
                st = tails[r]
                # the reference adds 1e-7 before dividing; the sum is O(500)
                # so the epsilon is ~1e-10 relative - far below bf16 noise.
                rcp = tail_pool.tile([128, 1], f32, tag="rcp", name="rcp")
                nc.vector.reciprocal(rcp[:], st["sb"][:])
                ot = tail_pool.tile([128, J16], f32, tag="ot", name="ot")
                nc.vector.tensor_scalar_mul(ot[:], st["e2"][:], rcp[:])
                nc.sync.dma_start(out[r, :, :], ot[:])
                del tails[r]

            # ---- prologue PE stream: warmup, then the bias matmuls ------
            warm(WARM_A)
            emit_bias()
            warm(WARM_B)

            # ---- main loop ----------------------------------------------
            for r in range(R):
                if r + 1 < R:
                    issue_xt_row(r + 1)
                if r > 0:
                    tail_exp(r - 1)
                sc_ps = init_scps(r)
                for gi, grp in enumerate(GRPS[r]):
                    y1s, y2s, y3s = {}, {}, {}
                    y1get = lambda c, k: y1s[(c, k)][:]
                    y2get = lambda c, k: y2s[c][:, k, :]
                    for c in grp:
                        emit_l1(r, c, y1s)
                    for c in grp:
                        emit_mid(h0t, y1get, r, c, y2s, "y2")
                    if gi == 0 and r > 0:
                        tail_sum(r - 1)
                    for c in grp:
                        emit_mid(h1t, y2get, r, c, y3s, "y3")
                    if gi == min(1, len(GRPS[r]) - 1) and r > 0:
                        tail_norm(r - 1)
                    if gi == 0:
                        emit_basecoat(r, sc_ps)
                    for c in grp:
                        emit_score(sc_ps, y3s, r, c)

            # final row's tail
            tail_exp(R - 1)
            tail_sum(R - 1)
            tail_norm(R - 1)
    nc.compile()
    return nc


def _get_nc(padts=None):
    if padts is None:
        padts = _CACHE.get("last_padts", (1152,) * R)
    padts = tuple(padts)
    _CACHE["last_padts"] = padts
    key = ("nc", padts)
    if key not in _CACHE:
        _CACHE[key] = _build(padts)
    return _CACHE[key]


def _prep(inputs):
    """Compact the masked-in tokens per row, sort rows by count so each
    row slot (shared across the 8 SPMD cores) pads only to its own max;
    returns (in_maps, order, gidx, cnt, padts)."""
    import ml_dtypes

    bf = ml_dtypes.bfloat16
    x = np.asarray(inputs["x"], dtype=np.float32)
    mask = np.asarray(inputs["mask"]).astype(bool)
    head_mask = mask[:, :TH]
    gidx = [np.nonzero(head_mask[b])[0] for b in range(B)]
    cnt = np.array([len(g) for g in gidx])
    order = np.argsort(-cnt, kind="stable")  # slot j <- ranks [8j, 8j+8)
    padts = tuple(
        max(16, int(np.ceil(max(int(cnt[order[NCORES * j]]), 1) / 16)) * 16)
        for j in range(R)
    )
    TC = max(padts)

    # xt[b] = x gathered + transposed to [128, KD, TC] (k-major, on host)
    xt = np.zeros((B, 128, KD * TC), dtype=bf)
    for b in range(B):
        g = x[b, gidx[b]].astype(bf)  # [cnt, D]
        # [cnt, D] -> [D, cnt] -> [KD, 128, cnt] -> [128, KD, cnt]
        t = g.T.reshape(KD, 128, len(gidx[b])).transpose(1, 0, 2)
        xt[b].reshape(128, KD, TC)[:, :, : cnt[b]] = t

    w1 = np.asarray(inputs["proj_head"], dtype=np.float32).astype(bf)
    # w1m[p, m, k, q] = w1[k*128+p, m*128+q]
    w1m = np.ascontiguousarray(
        w1.reshape(KD, 128, KP, 128).transpose(1, 2, 0, 3)
    )
    wpw = np.asarray(inputs["proj_prep"], dtype=np.float32).astype(bf)
    wcw = np.asarray(inputs["proj_child"], dtype=np.float32).astype(bf)
    hw = np.asarray(inputs["hidden_W"], dtype=np.float32).astype(bf)
    hhw = np.ascontiguousarray(hw.reshape(2 * P, P))
    scw = np.asarray(inputs["scorer"], dtype=np.float32).astype(bf)

    in_maps = []
    for i in range(NCORES):
        rows = [order[NCORES * j + i] for j in range(R)]
        # wpcx = wp | wc | prep/child cols | scorer col | mask-penalty cols
        wpcx = np.zeros((D, WPCX_COLS), dtype=bf)
        wpcx[:, 0:P] = wpw
        wpcx[:, P:XCOL] = wcw
        for j, b in enumerate(rows):
            wpcx[:, XCOL + 2 * j] = x[b, T - 2].astype(bf)
            wpcx[:, XCOL + 2 * j + 1] = x[b, T - 1].astype(bf)
        wpcx[0:P, SCOL] = scw[:, 0]
        # penalty columns: wpcx[k*128+p, PCOL+j] = pen(slot k, token j*128+p)
        pen = np.full((R, J16 * 128), PEN, dtype=np.float32)
        for j, b in enumerate(rows):
            pen[j, : cnt[b]] = 0.0
        pen_kpj = pen.reshape(R, J16, 128).transpose(0, 2, 1).reshape(
            R * 128, J16
        )
        wpcx[0 : R * 128, PCOL : PCOL + J16] = pen_kpj.astype(bf)
        in_maps.append(
            {
                "xs": np.ascontiguousarray(xt[rows]),
                "w1m": w1m,
                "wpcx": np.ascontiguousarray(wpcx),
                "hh": hhw,
            }
        )
    return in_maps, order, gidx, cnt, padts


def _run(inputs, **kwargs):
    from concourse.bass_utils import run_bass_kernel_spmd

    in_maps, order, gidx, cnt, padts = _prep(inputs)
    nc = _get_nc(padts)
    res = run_bass_kernel_spmd(
        nc, in_maps, core_ids=list(range(NCORES)), **kwargs
    )
    full = np.zeros((B, TH), dtype=np.float32)
    for i in range(NCORES):
        oc = res.results[i]["out"]  # [R, 128, J16]; token t = j*128+p
        for j in range(R):
            b = order[NCORES * j + i]
            full[b, gidx[b]] = oc[j].T.reshape(-1)[: cnt[b]]
    return full, res


def kernel(**inputs) -> np.ndarray:
    out, _ = _run(inputs)
    return out
